# revision 28
# baseline (speedup 1.0000x reference)
"""GroupedQueryAttention (B=1, S=2048, D=4096, 32 Q heads / 8 KV heads) on 8 TRN2 cores.

Sharding: one KV group (4 Q heads + 1 KV head) per core.  Per core:
  - QKV projection for its head group (q^T/k^T/v^T orientation: dims on partitions)
  - RoPE on q (scale folded in) and k via DVE
  - causal flash-style attention in scores^T orientation, issued chunk-by-chunk
    right after that chunk's QKV projection so ACT/DVE attention work overlaps
    the next chunk's PE-bound projection matmuls:
      scoresT[t,s] tiles from PE; causal mask applied on PE as an accumulating
      ident^T @ mask matmul into the same PSUM bank; exp on ACT -> P^T (bf16);
      denominator = bf16 DVE accum (2x mode) + ones-matmul partition reduce;
      AV accumulation out^T[d,s] on PE; normalize via reciprocal+ones-broadcast.
  - AllToAll swaps head-dims for seq-slices: each core ends with the FULL
    attention output (all 4096 dims) for its 256-row seq slice.
  - out-projection against full Wo -> out[256, 4096]; host concatenates + bias.

All matmuls bf16 inputs / fp32 PSUM accumulation.
"""
import numpy as np
import ml_dtypes

from concourse import bass, bacc, tile, mybir
from concourse.bass_utils import run_bass_kernel_spmd

BF16 = ml_dtypes.bfloat16
F32 = np.float32

D = 4096          # model dim
S = 2048          # sequence
NH = 32           # query heads
NG = 8            # kv heads == n cores
HD = 128          # head dim
G = NH // NG      # 4 query heads per group/core
KV = NG * HD      # 1024
BASE = 50000.0
SCALE = 1.0 / np.sqrt(HD)
N_CORES = 8
SC = S // 512     # 4 s-chunks of 512
MC = D // 128     # 32 contraction chunks
SSLICE = S // N_CORES  # 256 rows of final output per core

_CACHE = {}


def _build(reps: int = 1):
    f32 = mybir.dt.float32
    bf16 = mybir.dt.bfloat16

    nc = bacc.Bacc("TRN2", target_bir_lowering=False, debug=False,
                   num_devices=N_CORES)

    # ---- I/O ----
    xt_d = nc.dram_tensor("xt", [128, SC * MC * 512], bf16, kind="ExternalInput")
    wq_d = nc.dram_tensor("wq", [128, MC * 512], bf16, kind="ExternalInput")
    wk_d = nc.dram_tensor("wk", [128, MC * 128], bf16, kind="ExternalInput")
    wv_d = nc.dram_tensor("wv", [128, MC * 128], bf16, kind="ExternalInput")
    wo_d = nc.dram_tensor("wo", [128, 8 * MC * 512], bf16, kind="ExternalInput")
    cosq_d = nc.dram_tensor("cosq", [128, S], bf16, kind="ExternalInput")
    sinq_d = nc.dram_tensor("sinq", [128, S], bf16, kind="ExternalInput")
    cosk_d = nc.dram_tensor("cosk", [128, S], bf16, kind="ExternalInput")
    sink_d = nc.dram_tensor("sink", [128, S], bf16, kind="ExternalInput")
    bq_d = nc.dram_tensor("bq", [128, G], f32, kind="ExternalInput")
    bk_d = nc.dram_tensor("bk", [128, 1], f32, kind="ExternalInput")
    bv_d = nc.dram_tensor("bv", [128, 1], f32, kind="ExternalInput")
    mask_d = nc.dram_tensor("mask", [128, 4 * 512], bf16, kind="ExternalInput")
    ident_d = nc.dram_tensor("ident", [128, 128], bf16, kind="ExternalInput")
    onem_d = nc.dram_tensor("onem", [128, 128], bf16, kind="ExternalInput")
    out_d = nc.dram_tensor("out", [SSLICE, D], f32, kind="ExternalOutput")

    Ident = mybir.ActivationFunctionType.Identity
    CopyF = mybir.ActivationFunctionType.Copy
    Exp = mybir.ActivationFunctionType.Exp
    mult = mybir.AluOpType.mult

    with tile.TileContext(nc) as tc:
        with tc.tile_pool(name="const", bufs=1) as cp, \
             tc.tile_pool(name="pers", bufs=1) as pp, \
             tc.tile_pool(name="dram", bufs=1, space="DRAM") as dramp:
            # constants
            bq = cp.tile([128, G], f32); nc.sync.dma_start(bq[:], bq_d[:])
            bk = cp.tile([128, 1], f32); nc.sync.dma_start(bk[:], bk_d[:])
            bv = cp.tile([128, 1], f32); nc.sync.dma_start(bv[:], bv_d[:])
            mask = cp.tile([128, 4 * 512], bf16); nc.sync.dma_start(mask[:], mask_d[:])
            ident = cp.tile([128, 128], bf16); nc.sync.dma_start(ident[:], ident_d[:])
            onem = cp.tile([128, 128], bf16); nc.sync.dma_start(onem[:], onem_d[:])

            # persistent per-rep intermediates
            qT = pp.tile([128, G * S], bf16)      # rope'd q^T, head h at [:, h*S:]
            kT = pp.tile([128, S], bf16)
            vN = pp.tile([128, S], bf16)          # v natural, t-chunk tt at [:, tt*128:]
            attnT = pp.tile([128, G * S], bf16)   # attention out^T

            # tiny warmup collective: the first cc op after idle pays ~11us
            # trigger latency + cold links; burn that during the QKV phase so
            # the real AllToAlls run warm (content is irrelevant)
            warm_in = dramp.tile([N_CORES, 8], bf16, name="warm_in")
            warm_out = dramp.tile([N_CORES, 8], bf16, name="warm_out")
            nc.gpsimd.collective_compute(
                "AllToAll", mybir.AluOpType.bypass,
                replica_groups=[list(range(N_CORES))],
                ins=[warm_in.opt()], outs=[warm_out.opt()])

            for _rep in range(reps):
                # ====== fused QKV projection + attention, chunk by chunk ======
                with tc.tile_pool(name="wqkv", bufs=1) as wp, \
                     tc.tile_pool(name="xs", bufs=2) as xsp, \
                     tc.tile_pool(name="rtmp", bufs=3) as rtp, \
                     tc.tile_pool(name="es_sb", bufs=6) as esp, \
                     tc.tile_pool(name="acc_sb", bufs=2) as accp, \
                     tc.tile_pool(name="rope_ps", bufs=1, space="PSUM") as rpp, \
                     tc.tile_pool(name="qkv_ps", bufs=2, space="PSUM") as qps, \
                     tc.tile_pool(name="qk_ps", bufs=2, space="PSUM") as qkp, \
                     tc.tile_pool(name="av_ps", bufs=2, space="PSUM") as avp, \
                     tc.tile_pool(name="b_ps", bufs=1, space="PSUM") as bpp:
                    # first x chunk + wq, interleaved in 1MB pieces so the first
                    # projection matmul can start ~immediately
                    xs0 = xsp.tile([128, MC * 512], bf16, name="xs")
                    wq = wp.tile([128, MC * 512], bf16)
                    # fine-grained pieces (2 mc-blocks each) so arrival keeps
                    # pace with the first projection block's consumption
                    for g4 in range(16):
                        sl = slice(g4 * 2 * 512, (g4 + 1) * 2 * 512)
                        nc.sync.dma_start(xs0[:, sl], xt_d[:, sl])
                        nc.sync.dma_start(wq[:, sl], wq_d[:, sl])
                    # bulk loads not needed for several microseconds: hold them
                    # back (deps added below) so the first xs0/wq pieces get
                    # the full HBM bandwidth and PE can start early
                    held = []
                    wk = wp.tile([128, MC * 128], bf16)
                    held.append(nc.sync.dma_start(wk[:], wk_d[:]))
                    wv = wp.tile([128, MC * 128], bf16)
                    held.append(nc.sync.dma_start(wv[:], wv_d[:]))
                    cosq = wp.tile([128, S], bf16)
                    nc.sync.dma_start(cosq[:], cosq_d[:])
                    sinq = wp.tile([128, S], bf16)
                    nc.sync.dma_start(sinq[:], sinq_d[:])
                    cosk = wp.tile([128, S], bf16)
                    nc.sync.dma_start(cosk[:], cosk_d[:])
                    sink = wp.tile([128, S], bf16)
                    nc.sync.dma_start(sink[:], sink_d[:])
                    vTt = wp.tile([128, S], bf16)   # v^T (pre-transpose)
                    anchor = None

                    def rope(dst, ps, bias_ap, cos_t, sin_t, sc):
                        # dst = (b+ps)*cos + swap64(b+ps)*sin_signed  (all [128,512])
                        # partition-crossing reads must come from PSUM: walrus
                        # rejects SB+SB tensor_tensor with mismatched base partition
                        cs = slice(sc * 512, (sc + 1) * 512)
                        ps2 = rpp.tile([128, 512], f32, name="rps")
                        nc.scalar.activation(ps2[:], ps[:], Ident, bias=bias_ap)
                        t1 = rtp.tile([128, 512], f32, name="rope_t1")
                        nc.vector.tensor_tensor(t1[:], ps2[:], cos_t[:, cs], mult)
                        t2 = rtp.tile([128, 512], f32, name="rope_t2")
                        nc.vector.tensor_tensor(t2[0:64, :], ps2[64:128, :],
                                                sin_t[0:64, cs], mult)
                        nc.vector.tensor_tensor(t2[64:128, :], ps2[0:64, :],
                                                sin_t[64:128, cs], mult)
                        nc.vector.tensor_add(dst, t1[:], t2[:])

                    for sc in range(SC):
                        # ---- QKV projection for s-chunk sc ----
                        if sc == 0:
                            xs = xs0
                        else:
                            xs = xsp.tile([128, MC * 512], bf16, name="xs")
                            # 4 parallel queues: a single-queue 4MB DMA takes
                            # ~57us, which starves the next chunk
                            for q4 in range(4):
                                sl = slice(q4 * 8 * 512, (q4 + 1) * 8 * 512)
                                dx = nc.sync.dma_start(
                                    xs[:, sl],
                                    xt_d[:, sc * MC * 512 + q4 * 8 * 512:
                                         sc * MC * 512 + (q4 + 1) * 8 * 512])
                                if sc == 1 and anchor is not None:
                                    tile.add_dep_helper(
                                        dx.ins, anchor.ins, sync=True,
                                        reason="hold xs1 DMA for startup")
                        for blk in range(G + 2):  # 4 q heads, then k, then v
                            ps = qps.tile([128, 512], f32, name="qkv_ps")
                            for mc in range(MC):
                                if blk < G:
                                    lhsT = wq[:, mc * 512 + 128 * blk:
                                              mc * 512 + 128 * blk + 128]
                                elif blk == G:
                                    lhsT = wk[:, mc * 128:(mc + 1) * 128]
                                else:
                                    lhsT = wv[:, mc * 128:(mc + 1) * 128]
                                mm = nc.tensor.matmul(
                                    ps[:], lhsT,
                                    xs[:, mc * 512:(mc + 1) * 512],
                                    start=(mc == 0), stop=(mc == MC - 1))
                                if sc == 0 and blk == 0 and mc == 8:
                                    anchor = mm
                                    for hd_ in held:
                                        tile.add_dep_helper(
                                            hd_.ins, mm.ins, sync=True,
                                            reason="hold bulk DMA for startup")
                                    held = []
                            cs = slice(sc * 512, (sc + 1) * 512)
                            if blk < G:
                                rope(qT[:, blk * S + sc * 512:
                                         blk * S + sc * 512 + 512],
                                     ps, bq[:, blk:blk + 1], cosq, sinq, sc)
                            elif blk == G:
                                rope(kT[:, cs], ps, bk[:, 0:1], cosk, sink, sc)
                            else:
                                nc.scalar.activation(vTt[:, cs], ps[:], Ident,
                                                     bias=bv[:, 0:1])
                        # transpose this chunk's v^T -> v natural (4 t-tiles)
                        for tt in range(4 * sc, 4 * sc + 4):
                            pst = rpp.tile([128, 128], bf16, name="rps")
                            nc.tensor.transpose(pst[:], vTt[:, tt * 128:(tt + 1) * 128],
                                                ident[:])
                            nc.vector.tensor_copy(vN[:, tt * 128:(tt + 1) * 128], pst[:])

                        # ---- attention for s-chunk sc (queries of this chunk) ----
                        # diagonal tiles only compute the unmasked columns
                        # (s-local >= 128*r under the causal triangle)
                        ntt = 4 * (sc + 1)
                        for h in range(G):
                            qo = h * S + sc * 512
                            av = avp.tile([128, 512], f32, name="av")
                            acc = accp.tile([128, 512], bf16, name="acc")
                            for tt in range(ntt):
                                diag = tt >= 4 * sc
                                r = tt - 4 * sc if diag else 0
                                lo = 128 * r  # first live s-local column
                                qk = qkp.tile([128, 512], f32, name="qk")
                                nc.tensor.matmul(
                                    qk[:, lo:512],
                                    kT[:, tt * 128:(tt + 1) * 128],
                                    qT[:, qo + lo:qo + 512],
                                    start=True, stop=True)
                                es = esp.tile([128, 512], bf16, name="es")
                                nc.scalar.activation(
                                    es[:, lo:512], qk[:, lo:512], Exp)
                                if diag:
                                    # causal mask: es *= {0,1} pattern (cheap
                                    # bf16 2x-mode DVE; keeps mask work off
                                    # the pacing TensorE)
                                    nc.vector.tensor_tensor(
                                        es[:, lo:512], es[:, lo:512],
                                        mask[:, r * 512 + lo:(r + 1) * 512],
                                        mult)
                                if tt == 0:
                                    nc.vector.tensor_copy(acc[:], es[:])
                                else:
                                    nc.vector.tensor_add(
                                        acc[:, lo:512], acc[:, lo:512],
                                        es[:, lo:512])
                                nc.tensor.matmul(
                                    av[:, lo:512],
                                    vN[:, tt * 128:(tt + 1) * 128],
                                    es[:, lo:512],
                                    start=(tt == 0), stop=(tt == ntt - 1))
                            # denom: all-ones [128,128] matmul = partition
                            # reduce + broadcast in one standard-shape op
                            bps = bpp.tile([128, 512], f32, name="bps")
                            nc.tensor.matmul(bps[:], onem[:], acc[:],
                                             start=True, stop=True)
                            # drain the PSUM bank via a fast ACT copy so the
                            # next head's ones-matmul isn't blocked behind the
                            # 3.4us DVE reciprocal
                            den = esp.tile([128, 512], f32, name="den",
                                           bufs=2)
                            nc.scalar.activation(den[:], bps[:], CopyF)
                            recb = esp.tile([128, 512], f32, name="recb",
                                            bufs=2)
                            nc.vector.reciprocal(recb[:], den[:])
                            nc.vector.tensor_tensor(
                                attnT[:, h * S + sc * 512:h * S + sc * 512 + 512],
                                av[:], recb[:], mult)

                # ============ AllToAll (split per head, pipelined) ==========
                # one small a2a per head h, fired as soon as attn(3, h) is
                # normalized; each overlaps the attention tail / earlier
                # out-projection passes.
                bncs = []
                for h in range(G):
                    bi = dramp.tile([N_CORES * 128, SSLICE], bf16,
                                    name=f"bnc_in{h}")
                    bo_ = dramp.tile([N_CORES * 128, SSLICE], bf16,
                                     name=f"bnc_out{h}")
                    for j in range(N_CORES):
                        nc.sync.dma_start(
                            bi[128 * j:128 * j + 128, :],
                            attnT[:, h * S + SSLICE * j:
                                  h * S + SSLICE * j + SSLICE])
                    nc.gpsimd.collective_compute(
                        "AllToAll", mybir.AluOpType.bypass,
                        replica_groups=[list(range(N_CORES))],
                        ins=[bi.opt()], outs=[bo_.opt()])
                    bncs.append(bo_)

                # ============== out projection (four passes) ================
                # pass h accumulates the d-blocks mc = 4g + h (from collective
                # h): pass 0 seeds out_sb via ACT copy, passes 1-3 DVE-add.
                with tc.tile_pool(name="wo_sb", bufs=2) as wop, \
                     tc.tile_pool(name="wo_phase", bufs=1) as wph, \
                     tc.tile_pool(name="o_ps", bufs=3, space="PSUM") as ops:
                    attnF = wph.tile([128, MC * SSLICE], bf16)
                    for mc in range(MC):
                        g9, h = mc // G, mc % G
                        nc.sync.dma_start(
                            attnF[:, mc * SSLICE:(mc + 1) * SSLICE],
                            bncs[h][128 * g9:128 * g9 + 128, :])
                    out_sb = []
                    for s2 in range(2):
                        t = wph.tile([128, D], f32, name=f"out_sb{s2}")
                        out_sb.append(t)
                    NPP = MC // G  # 8 mc blocks per pass
                    for h in range(G):
                        mcs = [G * g9 + h for g9 in range(NPP)]
                        for e in range(8):
                            wo = wop.tile([128, NPP * 512], bf16, name="wo_t",
                                          bufs=3)
                            # 8 parallel DMA queues per chunk: one queue tops
                            # out ~70GB/s, consumption is ~230GB/s
                            base = (8 * h + e) * NPP * 512
                            for q4 in range(8):
                                sl = slice(q4 * NPP * 64, (q4 + 1) * NPP * 64)
                                nc.sync.dma_start(
                                    wo[:, sl],
                                    wo_d[:, base + q4 * NPP * 64:
                                         base + (q4 + 1) * NPP * 64])
                            for s2 in range(2):
                                po = ops.tile([128, 512], f32, name="po")
                                for i, mc in enumerate(mcs):
                                    nc.tensor.matmul(
                                        po[:],
                                        attnF[:, mc * SSLICE + 128 * s2:
                                              mc * SSLICE + 128 * s2 + 128],
                                        wo[:, i * 512:(i + 1) * 512],
                                        start=(i == 0), stop=(i == NPP - 1))
                                sl = slice(e * 512, (e + 1) * 512)
                                if h == 0:
                                    nc.scalar.activation(
                                        out_sb[s2][:, sl], po[:], CopyF)
                                else:
                                    nc.vector.tensor_add(
                                        out_sb[s2][:, sl], out_sb[s2][:, sl],
                                        po[:])
                                    if h == G - 1:
                                        nc.sync.dma_start(
                                            out_d[128 * s2:128 * (s2 + 1), sl],
                                            out_sb[s2][:, sl])
    nc.compile()
    return nc


def _prep_inputs(x, Wqkv, bqkv, Wo, bo):
    """Host-side shard prep. Returns in_maps for the 8 cores."""
    x0T = np.ascontiguousarray(np.asarray(x, F32)[0].T)          # [D, S]
    # [p, sc, mc, c] tiling of x^T
    xt_t = np.ascontiguousarray(
        x0T.reshape(MC, 128, SC, 512).transpose(1, 2, 0, 3)
    ).reshape(128, SC * MC * 512).astype(BF16)

    Wqkv = np.asarray(Wqkv, F32)
    Wo = np.asarray(Wo, F32)
    bqkv = np.asarray(bqkv, F32)

    # wo layout: [p, pass(4), e(8), mc-in-pass(8), 512] where pass = mc % 4,
    # matching the four-pass out-projection (pass h consumes collective h)
    wo4 = Wo.reshape(MC, 128, 8, 512)
    mc_order = [mc for h in range(4) for mc in range(MC) if mc % 4 == h]
    wo_t = np.ascontiguousarray(
        wo4[mc_order].reshape(4, MC // 4, 128, 8, 512).transpose(2, 0, 3, 1, 4)
    ).reshape(128, 8 * MC * 512).astype(BF16)

    # rope tables (transposed: [HD, S]); q tables fold in 1/sqrt(HD)
    inv_freq = 1.0 / (BASE ** (np.arange(0, HD, 2, dtype=np.float64) / HD))
    t = np.arange(S, dtype=np.float64)
    freqs = np.outer(t, inv_freq)                                # [S, 64]
    emb = np.concatenate([freqs, freqs], axis=1)                 # [S, HD]
    cosT = np.cos(emb).T.astype(F32)                             # [HD, S]
    sinT = np.sin(emb).T.astype(F32)
    sin_signed = np.concatenate([-sinT[:64], sinT[64:]], axis=0)
    cosq = np.ascontiguousarray(cosT * SCALE).astype(BF16)
    sinq = np.ascontiguousarray(sin_signed * SCALE).astype(BF16)
    cosk = np.ascontiguousarray(cosT).astype(BF16)
    sink = np.ascontiguousarray(sin_signed).astype(BF16)

    # binary causal mask patterns for the 4 diagonal t-tiles of each 512-wide
    # s-chunk (bf16 {0,1}: applied as a DVE multiply on the exp'd tile)
    p = np.arange(128)[:, None]
    f = np.arange(512)[None, :]
    mask = np.stack([np.where(128 * r + p <= f, 1.0, 0.0) for r in range(4)],
                    axis=1).astype(F32).reshape(128, 4 * 512).astype(BF16)

    ident = np.eye(128, dtype=np.float32).astype(BF16)
    onem = np.ones((128, 128), BF16)

    in_maps = []
    for g in range(N_CORES):
        wq_g = np.ascontiguousarray(
            Wqkv[:, 512 * g:512 * (g + 1)].reshape(MC, 128, 512)
            .transpose(1, 0, 2)).reshape(128, MC * 512).astype(BF16)
        wk_g = np.ascontiguousarray(
            Wqkv[:, D + 128 * g:D + 128 * (g + 1)].reshape(MC, 128, 128)
            .transpose(1, 0, 2)).reshape(128, MC * 128).astype(BF16)
        wv_g = np.ascontiguousarray(
            Wqkv[:, D + KV + 128 * g:D + KV + 128 * (g + 1)]
            .reshape(MC, 128, 128).transpose(1, 0, 2)
        ).reshape(128, MC * 128).astype(BF16)
        bq_g = np.ascontiguousarray(
            bqkv[512 * g:512 * (g + 1)].reshape(G, 128).T).astype(F32)
        bk_g = bqkv[D + 128 * g:D + 128 * (g + 1)].reshape(128, 1).astype(F32)
        bv_g = bqkv[D + KV + 128 * g:D + KV + 128 * (g + 1)].reshape(128, 1).astype(F32)
        in_maps.append({
            "xt": xt_t, "wq": wq_g, "wk": wk_g, "wv": wv_g, "wo": wo_t,
            "cosq": cosq, "sinq": sinq, "cosk": cosk, "sink": sink,
            "bq": bq_g, "bk": bk_g, "bv": bv_g, "mask": mask,
            "ident": ident, "onem": onem,
        })
    return in_maps


def kernel(x, Wqkv, bqkv, Wo, bo):
    if "nc" not in _CACHE:
        _CACHE["nc"] = _build(reps=1)
    nc = _CACHE["nc"]
    in_maps = _prep_inputs(x, Wqkv, bqkv, Wo, bo)
    res = run_bass_kernel_spmd(nc, in_maps, core_ids=list(range(N_CORES)))
    out = np.concatenate([res.results[g]["out"] for g in range(N_CORES)], axis=0)
    out = out + np.asarray(bo, F32)[None, :]
    return out[None].astype(F32)


# revision 29
# speedup vs baseline: 1.0836x; 1.0836x over previous
"""GroupedQueryAttention (B=1, S=2048, D=4096, 32 Q heads / 8 KV heads) on 8 TRN2 cores.

Sharding: one KV group (4 Q heads + 1 KV head) per core.  Per core:
  - QKV projection for its head group (q^T/k^T/v^T orientation: dims on partitions)
  - RoPE on q (scale folded in) and k via DVE
  - causal flash-style attention in scores^T orientation, issued chunk-by-chunk
    right after that chunk's QKV projection so ACT/DVE attention work overlaps
    the next chunk's PE-bound projection matmuls:
      scoresT[t,s] tiles from PE; causal mask applied on PE as an accumulating
      ident^T @ mask matmul into the same PSUM bank; exp on ACT -> P^T (bf16);
      denominator = bf16 DVE accum (2x mode) + ones-matmul partition reduce;
      AV accumulation out^T[d,s] on PE; normalize via reciprocal+ones-broadcast.
  - AllToAll swaps head-dims for seq-slices: each core ends with the FULL
    attention output (all 4096 dims) for its 256-row seq slice.
  - out-projection against full Wo -> out[256, 4096]; host concatenates + bias.

All matmuls bf16 inputs / fp32 PSUM accumulation.
"""
import numpy as np
import ml_dtypes

from concourse import bass, bacc, tile, mybir
from concourse.bass_utils import run_bass_kernel_spmd

BF16 = ml_dtypes.bfloat16
F32 = np.float32

D = 4096          # model dim
S = 2048          # sequence
NH = 32           # query heads
NG = 8            # kv heads == n cores
HD = 128          # head dim
G = NH // NG      # 4 query heads per group/core
KV = NG * HD      # 1024
BASE = 50000.0
SCALE = 1.0 / np.sqrt(HD)
N_CORES = 8
SC = S // 512     # 4 s-chunks of 512
MC = D // 128     # 32 contraction chunks
SSLICE = S // N_CORES  # 256 rows of final output per core

_CACHE = {}


def _build(reps: int = 1):
    f32 = mybir.dt.float32
    bf16 = mybir.dt.bfloat16

    nc = bacc.Bacc("TRN2", target_bir_lowering=False, debug=False,
                   num_devices=N_CORES)

    # ---- I/O ----
    xt_d = nc.dram_tensor("xt", [128, SC * MC * 512], bf16, kind="ExternalInput")
    wq_d = nc.dram_tensor("wq", [128, MC * 512], bf16, kind="ExternalInput")
    wk_d = nc.dram_tensor("wk", [128, MC * 128], bf16, kind="ExternalInput")
    wv_d = nc.dram_tensor("wv", [128, MC * 128], bf16, kind="ExternalInput")
    wo_d = nc.dram_tensor("wo", [128, 8 * MC * 512], bf16, kind="ExternalInput")
    cosq_d = nc.dram_tensor("cosq", [128, S], bf16, kind="ExternalInput")
    sinq_d = nc.dram_tensor("sinq", [128, S], bf16, kind="ExternalInput")
    cosk_d = nc.dram_tensor("cosk", [128, S], bf16, kind="ExternalInput")
    sink_d = nc.dram_tensor("sink", [128, S], bf16, kind="ExternalInput")
    bq_d = nc.dram_tensor("bq", [128, G], f32, kind="ExternalInput")
    bk_d = nc.dram_tensor("bk", [128, 1], f32, kind="ExternalInput")
    bv_d = nc.dram_tensor("bv", [128, 1], f32, kind="ExternalInput")
    mask_d = nc.dram_tensor("mask", [128, 4 * 512], bf16, kind="ExternalInput")
    ident_d = nc.dram_tensor("ident", [128, 128], bf16, kind="ExternalInput")
    onem_d = nc.dram_tensor("onem", [128, 128], bf16, kind="ExternalInput")
    out_d = nc.dram_tensor("out", [SSLICE, D], f32, kind="ExternalOutput")

    Ident = mybir.ActivationFunctionType.Identity
    CopyF = mybir.ActivationFunctionType.Copy
    Exp = mybir.ActivationFunctionType.Exp
    mult = mybir.AluOpType.mult

    with tile.TileContext(nc) as tc:
        with tc.tile_pool(name="const", bufs=1) as cp, \
             tc.tile_pool(name="pers", bufs=1) as pp, \
             tc.tile_pool(name="dram", bufs=1, space="DRAM") as dramp:
            # constants
            bq = cp.tile([128, G], f32); nc.sync.dma_start(bq[:], bq_d[:])
            bk = cp.tile([128, 1], f32); nc.sync.dma_start(bk[:], bk_d[:])
            bv = cp.tile([128, 1], f32); nc.sync.dma_start(bv[:], bv_d[:])
            mask = cp.tile([128, 4 * 512], bf16); nc.sync.dma_start(mask[:], mask_d[:])
            ident = cp.tile([128, 128], bf16); nc.sync.dma_start(ident[:], ident_d[:])
            onem = cp.tile([128, 128], bf16); nc.sync.dma_start(onem[:], onem_d[:])

            # persistent per-rep intermediates
            qT = pp.tile([128, G * S], bf16)      # rope'd q^T, head h at [:, h*S:]
            kT = pp.tile([128, S], bf16)
            vN = pp.tile([128, S], bf16)          # v natural, t-chunk tt at [:, tt*128:]
            attnT = pp.tile([128, G * S], bf16)   # attention out^T

            # tiny warmup collective: the first cc op after idle pays ~11us
            # trigger latency + cold links; burn that during the QKV phase so
            # the real AllToAlls run warm (content is irrelevant)
            warm_in = dramp.tile([N_CORES, 8], bf16, name="warm_in")
            warm_out = dramp.tile([N_CORES, 8], bf16, name="warm_out")
            nc.gpsimd.collective_compute(
                "AllToAll", mybir.AluOpType.bypass,
                replica_groups=[list(range(N_CORES))],
                ins=[warm_in.opt()], outs=[warm_out.opt()])

            for _rep in range(reps):
                # ====== fused QKV projection + attention, chunk by chunk ======
                with tc.tile_pool(name="wqkv", bufs=1) as wp, \
                     tc.tile_pool(name="xs", bufs=2) as xsp, \
                     tc.tile_pool(name="rtmp", bufs=3) as rtp, \
                     tc.tile_pool(name="es_sb", bufs=6) as esp, \
                     tc.tile_pool(name="acc_sb", bufs=2) as accp, \
                     tc.tile_pool(name="rope_ps", bufs=1, space="PSUM") as rpp, \
                     tc.tile_pool(name="qkv_ps", bufs=2, space="PSUM") as qps, \
                     tc.tile_pool(name="qk_ps", bufs=2, space="PSUM") as qkp, \
                     tc.tile_pool(name="av_ps", bufs=2, space="PSUM") as avp, \
                     tc.tile_pool(name="b_ps", bufs=1, space="PSUM") as bpp:
                    # first x chunk + wq, interleaved in 1MB pieces so the first
                    # projection matmul can start ~immediately
                    xs0 = xsp.tile([128, MC * 512], bf16, name="xs")
                    wq = wp.tile([128, MC * 512], bf16)
                    # fine-grained pieces (2 mc-blocks each) so arrival keeps
                    # pace with the first projection block's consumption
                    for g4 in range(16):
                        sl = slice(g4 * 2 * 512, (g4 + 1) * 2 * 512)
                        nc.sync.dma_start(xs0[:, sl], xt_d[:, sl])
                        nc.sync.dma_start(wq[:, sl], wq_d[:, sl])
                    # bulk loads not needed for several microseconds: hold them
                    # back (deps added below) so the first xs0/wq pieces get
                    # the full HBM bandwidth and PE can start early
                    held = []
                    wk = wp.tile([128, MC * 128], bf16)
                    held.append(nc.sync.dma_start(wk[:], wk_d[:]))
                    wv = wp.tile([128, MC * 128], bf16)
                    held.append(nc.sync.dma_start(wv[:], wv_d[:]))
                    cosq = wp.tile([128, S], bf16)
                    nc.sync.dma_start(cosq[:], cosq_d[:])
                    sinq = wp.tile([128, S], bf16)
                    nc.sync.dma_start(sinq[:], sinq_d[:])
                    cosk = wp.tile([128, S], bf16)
                    nc.sync.dma_start(cosk[:], cosk_d[:])
                    sink = wp.tile([128, S], bf16)
                    nc.sync.dma_start(sink[:], sink_d[:])
                    vTt = wp.tile([128, S], bf16)   # v^T (pre-transpose)
                    anchor = None

                    def rope(dst, ps, bias_ap, cos_t, sin_t, sc):
                        # dst = (b+ps)*cos + swap64(b+ps)*sin_signed  (all [128,512])
                        # partition-crossing reads must come from PSUM: walrus
                        # rejects SB+SB tensor_tensor with mismatched base partition
                        cs = slice(sc * 512, (sc + 1) * 512)
                        ps2 = rpp.tile([128, 512], f32, name="rps")
                        nc.scalar.activation(ps2[:], ps[:], Ident, bias=bias_ap)
                        t1 = rtp.tile([128, 512], f32, name="rope_t1")
                        nc.vector.tensor_tensor(t1[:], ps2[:], cos_t[:, cs], mult)
                        t2 = rtp.tile([128, 512], f32, name="rope_t2")
                        nc.vector.tensor_tensor(t2[0:64, :], ps2[64:128, :],
                                                sin_t[0:64, cs], mult)
                        nc.vector.tensor_tensor(t2[64:128, :], ps2[0:64, :],
                                                sin_t[64:128, cs], mult)
                        nc.vector.tensor_add(dst, t1[:], t2[:])

                    for sc in range(SC):
                        # ---- QKV projection for s-chunk sc ----
                        if sc == 0:
                            xs = xs0
                        else:
                            xs = xsp.tile([128, MC * 512], bf16, name="xs")
                            # 4 parallel queues: a single-queue 4MB DMA takes
                            # ~57us, which starves the next chunk
                            for q4 in range(4):
                                sl = slice(q4 * 8 * 512, (q4 + 1) * 8 * 512)
                                dx = nc.sync.dma_start(
                                    xs[:, sl],
                                    xt_d[:, sc * MC * 512 + q4 * 8 * 512:
                                         sc * MC * 512 + (q4 + 1) * 8 * 512])
                                if sc == 1 and anchor is not None:
                                    tile.add_dep_helper(
                                        dx.ins, anchor.ins, sync=True,
                                        reason="hold xs1 DMA for startup")
                        for blk in range(G + 2):  # 4 q heads, then k, then v
                            ps = qps.tile([128, 512], f32, name="qkv_ps")
                            for mc in range(MC):
                                if blk < G:
                                    lhsT = wq[:, mc * 512 + 128 * blk:
                                              mc * 512 + 128 * blk + 128]
                                elif blk == G:
                                    lhsT = wk[:, mc * 128:(mc + 1) * 128]
                                else:
                                    lhsT = wv[:, mc * 128:(mc + 1) * 128]
                                mm = nc.tensor.matmul(
                                    ps[:], lhsT,
                                    xs[:, mc * 512:(mc + 1) * 512],
                                    start=(mc == 0), stop=(mc == MC - 1))
                                if sc == 0 and blk == 0 and mc == 8:
                                    anchor = mm
                                    for hd_ in held:
                                        tile.add_dep_helper(
                                            hd_.ins, mm.ins, sync=True,
                                            reason="hold bulk DMA for startup")
                                    held = []
                            cs = slice(sc * 512, (sc + 1) * 512)
                            if blk < G:
                                rope(qT[:, blk * S + sc * 512:
                                         blk * S + sc * 512 + 512],
                                     ps, bq[:, blk:blk + 1], cosq, sinq, sc)
                            elif blk == G:
                                rope(kT[:, cs], ps, bk[:, 0:1], cosk, sink, sc)
                            else:
                                nc.scalar.activation(vTt[:, cs], ps[:], Ident,
                                                     bias=bv[:, 0:1])
                        # transpose this chunk's v^T -> v natural (4 t-tiles)
                        for tt in range(4 * sc, 4 * sc + 4):
                            pst = rpp.tile([128, 128], bf16, name="rps")
                            nc.tensor.transpose(pst[:], vTt[:, tt * 128:(tt + 1) * 128],
                                                ident[:])
                            nc.vector.tensor_copy(vN[:, tt * 128:(tt + 1) * 128], pst[:])

                        # ---- attention for s-chunk sc (queries of this chunk) ----
                        # diagonal tiles only compute the unmasked columns
                        # (s-local >= 128*r under the causal triangle)
                        ntt = 4 * (sc + 1)
                        for h in range(G):
                            qo = h * S + sc * 512
                            av = avp.tile([128, 512], f32, name="av")
                            acc = accp.tile([128, 512], bf16, name="acc")
                            for tt in range(ntt):
                                diag = tt >= 4 * sc
                                r = tt - 4 * sc if diag else 0
                                lo = 128 * r  # first live s-local column
                                qk = qkp.tile([128, 512], f32, name="qk")
                                nc.tensor.matmul(
                                    qk[:, lo:512],
                                    kT[:, tt * 128:(tt + 1) * 128],
                                    qT[:, qo + lo:qo + 512],
                                    start=True, stop=True)
                                es = esp.tile([128, 512], bf16, name="es")
                                nc.scalar.activation(
                                    es[:, lo:512], qk[:, lo:512], Exp)
                                if diag:
                                    # causal mask: es *= {0,1} pattern (cheap
                                    # bf16 2x-mode DVE; keeps mask work off
                                    # the pacing TensorE)
                                    nc.vector.tensor_tensor(
                                        es[:, lo:512], es[:, lo:512],
                                        mask[:, r * 512 + lo:(r + 1) * 512],
                                        mult)
                                if tt == 0:
                                    nc.vector.tensor_copy(acc[:], es[:])
                                else:
                                    nc.vector.tensor_add(
                                        acc[:, lo:512], acc[:, lo:512],
                                        es[:, lo:512])
                                nc.tensor.matmul(
                                    av[:, lo:512],
                                    vN[:, tt * 128:(tt + 1) * 128],
                                    es[:, lo:512],
                                    start=(tt == 0), stop=(tt == ntt - 1))
                            # denom: all-ones [128,128] matmul = partition
                            # reduce + broadcast in one standard-shape op
                            bps = bpp.tile([128, 512], f32, name="bps")
                            nc.tensor.matmul(bps[:], onem[:], acc[:],
                                             start=True, stop=True)
                            # drain the PSUM bank via a fast ACT copy so the
                            # next head's ones-matmul isn't blocked behind the
                            # 3.4us DVE reciprocal
                            den = esp.tile([128, 512], f32, name="den",
                                           bufs=2)
                            nc.scalar.activation(den[:], bps[:], CopyF)
                            recb = esp.tile([128, 512], f32, name="recb",
                                            bufs=2)
                            nc.vector.reciprocal(recb[:], den[:])
                            nc.vector.tensor_tensor(
                                attnT[:, h * S + sc * 512:h * S + sc * 512 + 512],
                                av[:], recb[:], mult)

                # ============ AllToAll (split per head, pipelined) ==========
                # one small a2a per head h, fired as soon as attn(3, h) is
                # normalized; each overlaps the attention tail / earlier
                # out-projection passes.
                bncs = []
                for h in range(G):
                    bi = dramp.tile([N_CORES * 128, SSLICE], bf16,
                                    name=f"bnc_in{h}")
                    bo_ = dramp.tile([N_CORES * 128, SSLICE], bf16,
                                     name=f"bnc_out{h}")
                    for j in range(N_CORES):
                        nc.sync.dma_start(
                            bi[128 * j:128 * j + 128, :],
                            attnT[:, h * S + SSLICE * j:
                                  h * S + SSLICE * j + SSLICE])
                    nc.gpsimd.collective_compute(
                        "AllToAll", mybir.AluOpType.bypass,
                        replica_groups=[list(range(N_CORES))],
                        ins=[bi.opt()], outs=[bo_.opt()])
                    bncs.append(bo_)

                # ============== out projection (four passes) ================
                # pass h accumulates the d-blocks mc = 4g + h (from collective
                # h): pass 0 seeds out_sb via ACT copy, passes 1-3 DVE-add.
                with tc.tile_pool(name="wo_sb", bufs=2) as wop, \
                     tc.tile_pool(name="wo_phase", bufs=1) as wph, \
                     tc.tile_pool(name="o_ps", bufs=3, space="PSUM") as ops:
                    attnF = wph.tile([128, MC * SSLICE], bf16)
                    for mc in range(MC):
                        g9, h = mc // G, mc % G
                        nc.sync.dma_start(
                            attnF[:, mc * SSLICE:(mc + 1) * SSLICE],
                            bncs[h][128 * g9:128 * g9 + 128, :])
                    out_sb = []
                    for s2 in range(2):
                        t = wph.tile([128, D], f32, name=f"out_sb{s2}")
                        out_sb.append(t)
                    NPP = MC // G  # 8 mc blocks per pass
                    for h in range(G):
                        mcs = [G * g9 + h for g9 in range(NPP)]
                        for e in range(8):
                            wo = wop.tile([128, NPP * 512], bf16, name="wo_t",
                                          bufs=3)
                            # 4 parallel DMA queues per chunk: one queue tops
                            # out ~70GB/s, consumption is ~230GB/s (8-way
                            # splitting regresses: queue congestion)
                            base = (8 * h + e) * NPP * 512
                            for q4 in range(4):
                                sl = slice(q4 * NPP * 128, (q4 + 1) * NPP * 128)
                                nc.sync.dma_start(
                                    wo[:, sl],
                                    wo_d[:, base + q4 * NPP * 128:
                                         base + (q4 + 1) * NPP * 128])
                            for s2 in range(2):
                                po = ops.tile([128, 512], f32, name="po")
                                for i, mc in enumerate(mcs):
                                    nc.tensor.matmul(
                                        po[:],
                                        attnF[:, mc * SSLICE + 128 * s2:
                                              mc * SSLICE + 128 * s2 + 128],
                                        wo[:, i * 512:(i + 1) * 512],
                                        start=(i == 0), stop=(i == NPP - 1))
                                sl = slice(e * 512, (e + 1) * 512)
                                if h == 0:
                                    nc.scalar.activation(
                                        out_sb[s2][:, sl], po[:], CopyF)
                                else:
                                    nc.vector.tensor_add(
                                        out_sb[s2][:, sl], out_sb[s2][:, sl],
                                        po[:])
                                    if h == G - 1:
                                        nc.sync.dma_start(
                                            out_d[128 * s2:128 * (s2 + 1), sl],
                                            out_sb[s2][:, sl])
    nc.compile()
    return nc


def _prep_inputs(x, Wqkv, bqkv, Wo, bo):
    """Host-side shard prep. Returns in_maps for the 8 cores."""
    x0T = np.ascontiguousarray(np.asarray(x, F32)[0].T)          # [D, S]
    # [p, sc, mc, c] tiling of x^T
    xt_t = np.ascontiguousarray(
        x0T.reshape(MC, 128, SC, 512).transpose(1, 2, 0, 3)
    ).reshape(128, SC * MC * 512).astype(BF16)

    Wqkv = np.asarray(Wqkv, F32)
    Wo = np.asarray(Wo, F32)
    bqkv = np.asarray(bqkv, F32)

    # wo layout: [p, pass(4), e(8), mc-in-pass(8), 512] where pass = mc % 4,
    # matching the four-pass out-projection (pass h consumes collective h)
    wo4 = Wo.reshape(MC, 128, 8, 512)
    mc_order = [mc for h in range(4) for mc in range(MC) if mc % 4 == h]
    wo_t = np.ascontiguousarray(
        wo4[mc_order].reshape(4, MC // 4, 128, 8, 512).transpose(2, 0, 3, 1, 4)
    ).reshape(128, 8 * MC * 512).astype(BF16)

    # rope tables (transposed: [HD, S]); q tables fold in 1/sqrt(HD)
    inv_freq = 1.0 / (BASE ** (np.arange(0, HD, 2, dtype=np.float64) / HD))
    t = np.arange(S, dtype=np.float64)
    freqs = np.outer(t, inv_freq)                                # [S, 64]
    emb = np.concatenate([freqs, freqs], axis=1)                 # [S, HD]
    cosT = np.cos(emb).T.astype(F32)                             # [HD, S]
    sinT = np.sin(emb).T.astype(F32)
    sin_signed = np.concatenate([-sinT[:64], sinT[64:]], axis=0)
    cosq = np.ascontiguousarray(cosT * SCALE).astype(BF16)
    sinq = np.ascontiguousarray(sin_signed * SCALE).astype(BF16)
    cosk = np.ascontiguousarray(cosT).astype(BF16)
    sink = np.ascontiguousarray(sin_signed).astype(BF16)

    # binary causal mask patterns for the 4 diagonal t-tiles of each 512-wide
    # s-chunk (bf16 {0,1}: applied as a DVE multiply on the exp'd tile)
    p = np.arange(128)[:, None]
    f = np.arange(512)[None, :]
    mask = np.stack([np.where(128 * r + p <= f, 1.0, 0.0) for r in range(4)],
                    axis=1).astype(F32).reshape(128, 4 * 512).astype(BF16)

    ident = np.eye(128, dtype=np.float32).astype(BF16)
    onem = np.ones((128, 128), BF16)

    in_maps = []
    for g in range(N_CORES):
        wq_g = np.ascontiguousarray(
            Wqkv[:, 512 * g:512 * (g + 1)].reshape(MC, 128, 512)
            .transpose(1, 0, 2)).reshape(128, MC * 512).astype(BF16)
        wk_g = np.ascontiguousarray(
            Wqkv[:, D + 128 * g:D + 128 * (g + 1)].reshape(MC, 128, 128)
            .transpose(1, 0, 2)).reshape(128, MC * 128).astype(BF16)
        wv_g = np.ascontiguousarray(
            Wqkv[:, D + KV + 128 * g:D + KV + 128 * (g + 1)]
            .reshape(MC, 128, 128).transpose(1, 0, 2)
        ).reshape(128, MC * 128).astype(BF16)
        bq_g = np.ascontiguousarray(
            bqkv[512 * g:512 * (g + 1)].reshape(G, 128).T).astype(F32)
        bk_g = bqkv[D + 128 * g:D + 128 * (g + 1)].reshape(128, 1).astype(F32)
        bv_g = bqkv[D + KV + 128 * g:D + KV + 128 * (g + 1)].reshape(128, 1).astype(F32)
        in_maps.append({
            "xt": xt_t, "wq": wq_g, "wk": wk_g, "wv": wv_g, "wo": wo_t,
            "cosq": cosq, "sinq": sinq, "cosk": cosk, "sink": sink,
            "bq": bq_g, "bk": bk_g, "bv": bv_g, "mask": mask,
            "ident": ident, "onem": onem,
        })
    return in_maps


def kernel(x, Wqkv, bqkv, Wo, bo):
    if "nc" not in _CACHE:
        _CACHE["nc"] = _build(reps=1)
    nc = _CACHE["nc"]
    in_maps = _prep_inputs(x, Wqkv, bqkv, Wo, bo)
    res = run_bass_kernel_spmd(nc, in_maps, core_ids=list(range(N_CORES)))
    out = np.concatenate([res.results[g]["out"] for g in range(N_CORES)], axis=0)
    out = out + np.asarray(bo, F32)[None, :]
    return out[None].astype(F32)


# revision 31
# speedup vs baseline: 1.0909x; 1.0067x over previous
"""GroupedQueryAttention (B=1, S=2048, D=4096, 32 Q heads / 8 KV heads) on 8 TRN2 cores.

Sharding: one KV group (4 Q heads + 1 KV head) per core.  Per core:
  - QKV projection for its head group (q^T/k^T/v^T orientation: dims on partitions)
  - RoPE on q (scale folded in) and k via DVE
  - causal flash-style attention in scores^T orientation, issued chunk-by-chunk
    right after that chunk's QKV projection so ACT/DVE attention work overlaps
    the next chunk's PE-bound projection matmuls:
      scoresT[t,s] tiles from PE; causal mask applied on PE as an accumulating
      ident^T @ mask matmul into the same PSUM bank; exp on ACT -> P^T (bf16);
      denominator = bf16 DVE accum (2x mode) + ones-matmul partition reduce;
      AV accumulation out^T[d,s] on PE; normalize via reciprocal+ones-broadcast.
  - AllToAll swaps head-dims for seq-slices: each core ends with the FULL
    attention output (all 4096 dims) for its 256-row seq slice.
  - out-projection against full Wo -> out[256, 4096]; host concatenates + bias.

All matmuls bf16 inputs / fp32 PSUM accumulation.
"""
import numpy as np
import ml_dtypes

from concourse import bass, bacc, tile, mybir
from concourse.bass_utils import run_bass_kernel_spmd

BF16 = ml_dtypes.bfloat16
F32 = np.float32

D = 4096          # model dim
S = 2048          # sequence
NH = 32           # query heads
NG = 8            # kv heads == n cores
HD = 128          # head dim
G = NH // NG      # 4 query heads per group/core
KV = NG * HD      # 1024
BASE = 50000.0
SCALE = 1.0 / np.sqrt(HD)
N_CORES = 8
SC = S // 512     # 4 s-chunks of 512
MC = D // 128     # 32 contraction chunks
SSLICE = S // N_CORES  # 256 rows of final output per core

_CACHE = {}


def _build(reps: int = 1):
    f32 = mybir.dt.float32
    bf16 = mybir.dt.bfloat16

    nc = bacc.Bacc("TRN2", target_bir_lowering=False, debug=False,
                   num_devices=N_CORES)

    # ---- I/O ----
    xt_d = nc.dram_tensor("xt", [128, SC * MC * 512], bf16, kind="ExternalInput")
    wq_d = nc.dram_tensor("wq", [128, MC * 512], bf16, kind="ExternalInput")
    wk_d = nc.dram_tensor("wk", [128, MC * 128], bf16, kind="ExternalInput")
    wv_d = nc.dram_tensor("wv", [128, MC * 128], bf16, kind="ExternalInput")
    wo_d = nc.dram_tensor("wo", [128, 8 * MC * 512], bf16, kind="ExternalInput")
    cosq_d = nc.dram_tensor("cosq", [128, S], bf16, kind="ExternalInput")
    sinq_d = nc.dram_tensor("sinq", [128, S], bf16, kind="ExternalInput")
    cosk_d = nc.dram_tensor("cosk", [128, S], bf16, kind="ExternalInput")
    sink_d = nc.dram_tensor("sink", [128, S], bf16, kind="ExternalInput")
    bq_d = nc.dram_tensor("bq", [128, G], f32, kind="ExternalInput")
    bk_d = nc.dram_tensor("bk", [128, 1], f32, kind="ExternalInput")
    bv_d = nc.dram_tensor("bv", [128, 1], f32, kind="ExternalInput")
    mask_d = nc.dram_tensor("mask", [128, 4 * 512], bf16, kind="ExternalInput")
    ident_d = nc.dram_tensor("ident", [128, 128], bf16, kind="ExternalInput")
    onem_d = nc.dram_tensor("onem", [128, 128], bf16, kind="ExternalInput")
    out_d = nc.dram_tensor("out", [SSLICE, D], f32, kind="ExternalOutput")

    Ident = mybir.ActivationFunctionType.Identity
    CopyF = mybir.ActivationFunctionType.Copy
    Exp = mybir.ActivationFunctionType.Exp
    mult = mybir.AluOpType.mult

    with tile.TileContext(nc) as tc:
        with tc.tile_pool(name="const", bufs=1) as cp, \
             tc.tile_pool(name="pers", bufs=1) as pp, \
             tc.tile_pool(name="dram", bufs=1, space="DRAM") as dramp:
            # constants
            bq = cp.tile([128, G], f32); nc.sync.dma_start(bq[:], bq_d[:])
            bk = cp.tile([128, 1], f32); nc.sync.dma_start(bk[:], bk_d[:])
            bv = cp.tile([128, 1], f32); nc.sync.dma_start(bv[:], bv_d[:])
            mask = cp.tile([128, 4 * 512], bf16); nc.sync.dma_start(mask[:], mask_d[:])
            ident = cp.tile([128, 128], bf16); nc.sync.dma_start(ident[:], ident_d[:])
            onem = cp.tile([128, 128], bf16); nc.sync.dma_start(onem[:], onem_d[:])

            # persistent per-rep intermediates
            qT = pp.tile([128, G * S], bf16)      # rope'd q^T, head h at [:, h*S:]
            kT = pp.tile([128, S], bf16)
            vN = pp.tile([128, S], bf16)          # v natural, t-chunk tt at [:, tt*128:]
            attnT = pp.tile([128, G * S], bf16)   # attention out^T

            # tiny warmup collective: the first cc op after idle pays ~11us
            # trigger latency + cold links; burn that during the QKV phase so
            # the real AllToAlls run warm (content is irrelevant)
            warm_in = dramp.tile([N_CORES, 8], bf16, name="warm_in")
            warm_out = dramp.tile([N_CORES, 8], bf16, name="warm_out")
            nc.gpsimd.collective_compute(
                "AllToAll", mybir.AluOpType.bypass,
                replica_groups=[list(range(N_CORES))],
                ins=[warm_in.opt()], outs=[warm_out.opt()])

            for _rep in range(reps):
                # ====== fused QKV projection + attention, chunk by chunk ======
                with tc.tile_pool(name="wqkv", bufs=1) as wp, \
                     tc.tile_pool(name="xs", bufs=2) as xsp, \
                     tc.tile_pool(name="rtmp", bufs=3) as rtp, \
                     tc.tile_pool(name="es_sb", bufs=8) as esp, \
                     tc.tile_pool(name="acc_sb", bufs=3) as accp, \
                     tc.tile_pool(name="rope_ps", bufs=1, space="PSUM") as rpp, \
                     tc.tile_pool(name="qkv_ps", bufs=2, space="PSUM") as qps, \
                     tc.tile_pool(name="qk_ps", bufs=2, space="PSUM") as qkp, \
                     tc.tile_pool(name="av_ps", bufs=2, space="PSUM") as avp, \
                     tc.tile_pool(name="b_ps", bufs=1, space="PSUM") as bpp:
                    # first x chunk + wq, interleaved in 1MB pieces so the first
                    # projection matmul can start ~immediately
                    xs0 = xsp.tile([128, MC * 512], bf16, name="xs")
                    wq = wp.tile([128, MC * 512], bf16)
                    # fine-grained pieces (2 mc-blocks each) so arrival keeps
                    # pace with the first projection block's consumption
                    for g4 in range(16):
                        sl = slice(g4 * 2 * 512, (g4 + 1) * 2 * 512)
                        nc.sync.dma_start(xs0[:, sl], xt_d[:, sl])
                        nc.sync.dma_start(wq[:, sl], wq_d[:, sl])
                    # bulk loads not needed for several microseconds: hold them
                    # back (deps added below) so the first xs0/wq pieces get
                    # the full HBM bandwidth and PE can start early
                    held = []
                    wk = wp.tile([128, MC * 128], bf16)
                    held.append(nc.sync.dma_start(wk[:], wk_d[:]))
                    wv = wp.tile([128, MC * 128], bf16)
                    held.append(nc.sync.dma_start(wv[:], wv_d[:]))
                    cosq = wp.tile([128, S], bf16)
                    nc.sync.dma_start(cosq[:], cosq_d[:])
                    sinq = wp.tile([128, S], bf16)
                    nc.sync.dma_start(sinq[:], sinq_d[:])
                    cosk = wp.tile([128, S], bf16)
                    nc.sync.dma_start(cosk[:], cosk_d[:])
                    sink = wp.tile([128, S], bf16)
                    nc.sync.dma_start(sink[:], sink_d[:])
                    vTt = wp.tile([128, S], bf16)   # v^T (pre-transpose)
                    anchor = None

                    def rope(dst, ps, bias_ap, cos_t, sin_t, sc):
                        # dst = (b+ps)*cos + swap64(b+ps)*sin_signed  (all [128,512])
                        # partition-crossing reads must come from PSUM: walrus
                        # rejects SB+SB tensor_tensor with mismatched base partition
                        cs = slice(sc * 512, (sc + 1) * 512)
                        ps2 = rpp.tile([128, 512], f32, name="rps")
                        nc.scalar.activation(ps2[:], ps[:], Ident, bias=bias_ap)
                        t1 = rtp.tile([128, 512], f32, name="rope_t1")
                        nc.vector.tensor_tensor(t1[:], ps2[:], cos_t[:, cs], mult)
                        t2 = rtp.tile([128, 512], f32, name="rope_t2")
                        nc.vector.tensor_tensor(t2[0:64, :], ps2[64:128, :],
                                                sin_t[0:64, cs], mult)
                        nc.vector.tensor_tensor(t2[64:128, :], ps2[0:64, :],
                                                sin_t[64:128, cs], mult)
                        nc.vector.tensor_add(dst, t1[:], t2[:])

                    for sc in range(SC):
                        # ---- QKV projection for s-chunk sc ----
                        if sc == 0:
                            xs = xs0
                        else:
                            xs = xsp.tile([128, MC * 512], bf16, name="xs")
                            # 4 parallel queues: a single-queue 4MB DMA takes
                            # ~57us, which starves the next chunk
                            for q4 in range(4):
                                sl = slice(q4 * 8 * 512, (q4 + 1) * 8 * 512)
                                dx = nc.sync.dma_start(
                                    xs[:, sl],
                                    xt_d[:, sc * MC * 512 + q4 * 8 * 512:
                                         sc * MC * 512 + (q4 + 1) * 8 * 512])
                                if sc == 1 and anchor is not None:
                                    tile.add_dep_helper(
                                        dx.ins, anchor.ins, sync=True,
                                        reason="hold xs1 DMA for startup")
                        for blk in range(G + 2):  # 4 q heads, then k, then v
                            ps = qps.tile([128, 512], f32, name="qkv_ps")
                            for mc in range(MC):
                                if blk < G:
                                    lhsT = wq[:, mc * 512 + 128 * blk:
                                              mc * 512 + 128 * blk + 128]
                                elif blk == G:
                                    lhsT = wk[:, mc * 128:(mc + 1) * 128]
                                else:
                                    lhsT = wv[:, mc * 128:(mc + 1) * 128]
                                mm = nc.tensor.matmul(
                                    ps[:], lhsT,
                                    xs[:, mc * 512:(mc + 1) * 512],
                                    start=(mc == 0), stop=(mc == MC - 1))
                                if sc == 0 and blk == 0 and mc == 2:
                                    anchor = mm
                                    for hd_ in held:
                                        tile.add_dep_helper(
                                            hd_.ins, mm.ins, sync=True,
                                            reason="hold bulk DMA for startup")
                                    held = []
                            cs = slice(sc * 512, (sc + 1) * 512)
                            if blk < G:
                                rope(qT[:, blk * S + sc * 512:
                                         blk * S + sc * 512 + 512],
                                     ps, bq[:, blk:blk + 1], cosq, sinq, sc)
                            elif blk == G:
                                rope(kT[:, cs], ps, bk[:, 0:1], cosk, sink, sc)
                            else:
                                nc.scalar.activation(vTt[:, cs], ps[:], Ident,
                                                     bias=bv[:, 0:1])
                        # transpose this chunk's v^T -> v natural (4 t-tiles)
                        for tt in range(4 * sc, 4 * sc + 4):
                            pst = rpp.tile([128, 128], bf16, name="rps")
                            nc.tensor.transpose(pst[:], vTt[:, tt * 128:(tt + 1) * 128],
                                                ident[:])
                            nc.vector.tensor_copy(vN[:, tt * 128:(tt + 1) * 128], pst[:])

                        # ---- attention for s-chunk sc (queries of this chunk) ----
                        # diagonal tiles only compute the unmasked columns
                        # (s-local >= 128*r under the causal triangle)
                        ntt = 4 * (sc + 1)
                        for h in range(G):
                            qo = h * S + sc * 512
                            av = avp.tile([128, 512], f32, name="av")
                            acc = accp.tile([128, 512], bf16, name="acc")
                            for tt in range(ntt):
                                diag = tt >= 4 * sc
                                r = tt - 4 * sc if diag else 0
                                lo = 128 * r  # first live s-local column
                                qk = qkp.tile([128, 512], f32, name="qk")
                                nc.tensor.matmul(
                                    qk[:, lo:512],
                                    kT[:, tt * 128:(tt + 1) * 128],
                                    qT[:, qo + lo:qo + 512],
                                    start=True, stop=True)
                                es = esp.tile([128, 512], bf16, name="es")
                                nc.scalar.activation(
                                    es[:, lo:512], qk[:, lo:512], Exp)
                                if diag:
                                    # causal mask: es *= {0,1} pattern (cheap
                                    # bf16 2x-mode DVE; keeps mask work off
                                    # the pacing TensorE)
                                    nc.vector.tensor_tensor(
                                        es[:, lo:512], es[:, lo:512],
                                        mask[:, r * 512 + lo:(r + 1) * 512],
                                        mult)
                                if tt == 0:
                                    nc.vector.tensor_copy(acc[:], es[:])
                                else:
                                    nc.vector.tensor_add(
                                        acc[:, lo:512], acc[:, lo:512],
                                        es[:, lo:512])
                                nc.tensor.matmul(
                                    av[:, lo:512],
                                    vN[:, tt * 128:(tt + 1) * 128],
                                    es[:, lo:512],
                                    start=(tt == 0), stop=(tt == ntt - 1))
                            # denom: all-ones [128,128] matmul = partition
                            # reduce + broadcast in one standard-shape op
                            bps = bpp.tile([128, 512], f32, name="bps")
                            nc.tensor.matmul(bps[:], onem[:], acc[:],
                                             start=True, stop=True)
                            # drain the PSUM bank via a fast ACT copy so the
                            # next head's ones-matmul isn't blocked behind the
                            # 3.4us DVE reciprocal
                            den = esp.tile([128, 512], f32, name="den",
                                           bufs=2)
                            nc.scalar.activation(den[:], bps[:], CopyF)
                            recb = esp.tile([128, 512], f32, name="recb",
                                            bufs=2)
                            nc.vector.reciprocal(recb[:], den[:])
                            nc.vector.tensor_tensor(
                                attnT[:, h * S + sc * 512:h * S + sc * 512 + 512],
                                av[:], recb[:], mult)

                # ============ AllToAll (split per head, pipelined) ==========
                # one small a2a per head h, fired as soon as attn(3, h) is
                # normalized; each overlaps the attention tail / earlier
                # out-projection passes.
                bncs = []
                for h in range(G):
                    bi = dramp.tile([N_CORES * 128, SSLICE], bf16,
                                    name=f"bnc_in{h}")
                    bo_ = dramp.tile([N_CORES * 128, SSLICE], bf16,
                                     name=f"bnc_out{h}")
                    for j in range(N_CORES):
                        nc.sync.dma_start(
                            bi[128 * j:128 * j + 128, :],
                            attnT[:, h * S + SSLICE * j:
                                  h * S + SSLICE * j + SSLICE])
                    nc.gpsimd.collective_compute(
                        "AllToAll", mybir.AluOpType.bypass,
                        replica_groups=[list(range(N_CORES))],
                        ins=[bi.opt()], outs=[bo_.opt()])
                    bncs.append(bo_)

                # ============== out projection (four passes) ================
                # pass h accumulates the d-blocks mc = 4g + h (from collective
                # h): pass 0 seeds out_sb via ACT copy, passes 1-3 DVE-add.
                with tc.tile_pool(name="wo_sb", bufs=2) as wop, \
                     tc.tile_pool(name="wo_phase", bufs=1) as wph, \
                     tc.tile_pool(name="o_ps", bufs=3, space="PSUM") as ops:
                    attnF = wph.tile([128, MC * SSLICE], bf16)
                    for mc in range(MC):
                        g9, h = mc // G, mc % G
                        nc.sync.dma_start(
                            attnF[:, mc * SSLICE:(mc + 1) * SSLICE],
                            bncs[h][128 * g9:128 * g9 + 128, :])
                    out_sb = []
                    for s2 in range(2):
                        t = wph.tile([128, D], f32, name=f"out_sb{s2}")
                        out_sb.append(t)
                    NPP = MC // G  # 8 mc blocks per pass
                    for h in range(G):
                        mcs = [G * g9 + h for g9 in range(NPP)]
                        for e in range(8):
                            wo = wop.tile([128, NPP * 512], bf16, name="wo_t",
                                          bufs=3)
                            # 4 parallel DMA queues per chunk: one queue tops
                            # out ~70GB/s, consumption is ~230GB/s (8-way
                            # splitting regresses: queue congestion)
                            base = (8 * h + e) * NPP * 512
                            for q4 in range(4):
                                sl = slice(q4 * NPP * 128, (q4 + 1) * NPP * 128)
                                nc.sync.dma_start(
                                    wo[:, sl],
                                    wo_d[:, base + q4 * NPP * 128:
                                         base + (q4 + 1) * NPP * 128])
                            for s2 in range(2):
                                po = ops.tile([128, 512], f32, name="po")
                                for i, mc in enumerate(mcs):
                                    nc.tensor.matmul(
                                        po[:],
                                        attnF[:, mc * SSLICE + 128 * s2:
                                              mc * SSLICE + 128 * s2 + 128],
                                        wo[:, i * 512:(i + 1) * 512],
                                        start=(i == 0), stop=(i == NPP - 1))
                                sl = slice(e * 512, (e + 1) * 512)
                                if h == 0:
                                    nc.scalar.activation(
                                        out_sb[s2][:, sl], po[:], CopyF)
                                else:
                                    nc.vector.tensor_add(
                                        out_sb[s2][:, sl], out_sb[s2][:, sl],
                                        po[:])
                                    if h == G - 1:
                                        nc.sync.dma_start(
                                            out_d[128 * s2:128 * (s2 + 1), sl],
                                            out_sb[s2][:, sl])
    nc.compile()
    return nc


def _prep_inputs(x, Wqkv, bqkv, Wo, bo):
    """Host-side shard prep. Returns in_maps for the 8 cores."""
    x0T = np.ascontiguousarray(np.asarray(x, F32)[0].T)          # [D, S]
    # [p, sc, mc, c] tiling of x^T
    xt_t = np.ascontiguousarray(
        x0T.reshape(MC, 128, SC, 512).transpose(1, 2, 0, 3)
    ).reshape(128, SC * MC * 512).astype(BF16)

    Wqkv = np.asarray(Wqkv, F32)
    Wo = np.asarray(Wo, F32)
    bqkv = np.asarray(bqkv, F32)

    # wo layout: [p, pass(4), e(8), mc-in-pass(8), 512] where pass = mc % 4,
    # matching the four-pass out-projection (pass h consumes collective h)
    wo4 = Wo.reshape(MC, 128, 8, 512)
    mc_order = [mc for h in range(4) for mc in range(MC) if mc % 4 == h]
    wo_t = np.ascontiguousarray(
        wo4[mc_order].reshape(4, MC // 4, 128, 8, 512).transpose(2, 0, 3, 1, 4)
    ).reshape(128, 8 * MC * 512).astype(BF16)

    # rope tables (transposed: [HD, S]); q tables fold in 1/sqrt(HD)
    inv_freq = 1.0 / (BASE ** (np.arange(0, HD, 2, dtype=np.float64) / HD))
    t = np.arange(S, dtype=np.float64)
    freqs = np.outer(t, inv_freq)                                # [S, 64]
    emb = np.concatenate([freqs, freqs], axis=1)                 # [S, HD]
    cosT = np.cos(emb).T.astype(F32)                             # [HD, S]
    sinT = np.sin(emb).T.astype(F32)
    sin_signed = np.concatenate([-sinT[:64], sinT[64:]], axis=0)
    cosq = np.ascontiguousarray(cosT * SCALE).astype(BF16)
    sinq = np.ascontiguousarray(sin_signed * SCALE).astype(BF16)
    cosk = np.ascontiguousarray(cosT).astype(BF16)
    sink = np.ascontiguousarray(sin_signed).astype(BF16)

    # binary causal mask patterns for the 4 diagonal t-tiles of each 512-wide
    # s-chunk (bf16 {0,1}: applied as a DVE multiply on the exp'd tile)
    p = np.arange(128)[:, None]
    f = np.arange(512)[None, :]
    mask = np.stack([np.where(128 * r + p <= f, 1.0, 0.0) for r in range(4)],
                    axis=1).astype(F32).reshape(128, 4 * 512).astype(BF16)

    ident = np.eye(128, dtype=np.float32).astype(BF16)
    onem = np.ones((128, 128), BF16)

    in_maps = []
    for g in range(N_CORES):
        wq_g = np.ascontiguousarray(
            Wqkv[:, 512 * g:512 * (g + 1)].reshape(MC, 128, 512)
            .transpose(1, 0, 2)).reshape(128, MC * 512).astype(BF16)
        wk_g = np.ascontiguousarray(
            Wqkv[:, D + 128 * g:D + 128 * (g + 1)].reshape(MC, 128, 128)
            .transpose(1, 0, 2)).reshape(128, MC * 128).astype(BF16)
        wv_g = np.ascontiguousarray(
            Wqkv[:, D + KV + 128 * g:D + KV + 128 * (g + 1)]
            .reshape(MC, 128, 128).transpose(1, 0, 2)
        ).reshape(128, MC * 128).astype(BF16)
        bq_g = np.ascontiguousarray(
            bqkv[512 * g:512 * (g + 1)].reshape(G, 128).T).astype(F32)
        bk_g = bqkv[D + 128 * g:D + 128 * (g + 1)].reshape(128, 1).astype(F32)
        bv_g = bqkv[D + KV + 128 * g:D + KV + 128 * (g + 1)].reshape(128, 1).astype(F32)
        in_maps.append({
            "xt": xt_t, "wq": wq_g, "wk": wk_g, "wv": wv_g, "wo": wo_t,
            "cosq": cosq, "sinq": sinq, "cosk": cosk, "sink": sink,
            "bq": bq_g, "bk": bk_g, "bv": bv_g, "mask": mask,
            "ident": ident, "onem": onem,
        })
    return in_maps


def kernel(x, Wqkv, bqkv, Wo, bo):
    if "nc" not in _CACHE:
        _CACHE["nc"] = _build(reps=1)
    nc = _CACHE["nc"]
    in_maps = _prep_inputs(x, Wqkv, bqkv, Wo, bo)
    res = run_bass_kernel_spmd(nc, in_maps, core_ids=list(range(N_CORES)))
    out = np.concatenate([res.results[g]["out"] for g in range(N_CORES)], axis=0)
    out = out + np.asarray(bo, F32)[None, :]
    return out[None].astype(F32)
